# revision 32
# baseline (speedup 1.0000x reference)
"""
Trainium2 Bass kernel for nn_LinearLUT (residual-binarized LUT linear layer).

Math restructure
----------------
reference(x) computes, per sample b and per table t (t = o*128 + j, one table
per (out_feature o, in_feature j)):

  table_out[b,t] = sum_l f_t(m_l * s_l[b, idx_1(t)], ..., m_l * s_l[b, idx_4(t)])

where f_t is the multilinear (Lagrange) interpolation of the 16-entry LUT
weight[t, :] on {-1,+1}^4, s_l are the level-l sign bits of x, and
idx_i(t) = input_mask[t*4+i].  Since every argument is +-m_l, f_t only
depends on the 4 sign bits => precompute (host, weight-static):

  Q_l[t, v] = sum_c weight[t,c] * prod_i (1 + m_l*sig(v,i)*tt(c,i))/2

a 16-entry lookup per (t, level), indexed by the 4-bit sign code
  code_l[b,t] = sum_i 2^i * bit_l[b, idx_i(t)]  =  (bit_l @ G)[b,t]
with G[j,t] = sum_i 2^i [idx_i(t)==j]  -- ONE matmul per level.

Step-basis LUT evaluation (Abel summation):
  Q_l[t, c] = Q_l[t, 0] + sum_{v=1..15} dQ_l[t,v] * [c >= v]
so each basis plane [c >= v] is ONE instruction on any of three engines:
  DVE:  tensor_scalar is_ge               (0/1 plane, coeff dQ)
  ACT:  activation Sign, bias 0.5-v       (+-1 plane, coeff dQ/2, consts
                                           folded into cvec)
  POOL: tensor_scalar is_ge               (0/1 plane, coeff dQ)
The 30 planes (2 levels x 15 thresholds) are split across DVE/ACT/POOL to
balance engine busy time; the per-output-feature segment sum is fused into
PSUM-accumulated N=1 matmuls (j-contraction) as before.

On device (per core; tables sharded 8 ways, T_C=2048 tables = 16 out
features per core):
  1. sign bits from xT (DVE, 3 small ops)
  2. codeT[t_p,(tile,b)] = G_chunk^T @ bitT   (PE, 32 matmuls)
  3. PSUM->SBUF fp16 copies, [128,1024] x2 per level (ACT)
  4. step planes on DVE/ACT/POOL per the assignment
  5. LUT-eval + segment-sum fused into PE: y[b,o] += plane^T @ dq_col
     (N=1 matmuls, PSUM-accumulated; seeded by rank-1 cvec matmul)
  6. y PSUM->SBUF on DVE, DMA out [128, 16] f32; host concatenates cores.
"""

import numpy as np

import concourse.bass as bass
import concourse.bacc as bacc
import concourse.mybir as mybir
import concourse.tile as tile
from concourse.bass_utils import run_bass_kernel_spmd

# Problem dims (hardcoded per contract)
LEVELS = 2
K = 4
KK = 16
IN = 128
OUT = 128
B = 128
T = IN * OUT  # 16384
NCORES = 8
T_C = T // NCORES     # 2048 tables per core
OL = OUT // NCORES    # 16 out features per core
NTILE = T_C // 128    # 16 t-tiles per core
NV = KK - 1           # 15 step thresholds v=1..15 (v=0 folded into cvec)

F16 = mybir.dt.float16
F32 = mybir.dt.float32

# ---- plane -> engine assignment (tuned against the CoreSim cost model) ----
# 'D' = DVE is_ge (0/1), 'A' = ACT Sign (+-1, coeff/2), 'P' = POOL is_ge.
ACT_PLANES = {(0, 10), (0, 11), (1, 12), (1, 13)}
POOL_PLANES = {(0, 12), (0, 13), (0, 14), (0, 15), (1, 14), (1, 15)}
# final plane: split between ACT (tiles < SPLIT_TILE) and POOL (rest) so all
# three engines finish together
SPLIT_PLANE = (1, 11)
SPLIT_TILE = 6


def _plane_tile_engine(l, v, tile):
    """Engine that computes basis plane (l, v) for tile `tile`."""
    if (l, v) == SPLIT_PLANE:
        return "A" if tile < SPLIT_TILE else "P"
    if (l, v) in ACT_PLANES:
        return "A"
    if (l, v) in POOL_PLANES:
        return "P"
    return "D"


def _plane_emit_order():
    """Interleave (l, v, engine) so each engine's queue is busy end-to-end
    and PE consumes planes roughly in completion order.  Each engine's own
    planes are emitted level-0 first (level-1 code is ready later)."""
    per_eng = {"D": [], "A": [], "P": []}
    for l in range(LEVELS):
        for v in range(1, KK):
            if (l, v) == SPLIT_PLANE:
                continue  # emitted last, split across ACT and POOL
            per_eng[_plane_tile_engine(l, v, 0)].append((l, v))
    # estimated per-plane engine cost (ns) and start offsets
    cost = {"D": 594.0, "A": 1892.0, "P": 1707.0}
    start = {"D": 4580.0, "A": 7700.0, "P": 4680.0}
    lvl_ready = {0: 4580.0, 1: 7700.0}
    order = []
    t_eng = dict(start)
    idx = {e: 0 for e in per_eng}
    while any(idx[e] < len(per_eng[e]) for e in per_eng):
        # pick engine whose next plane completes earliest
        best, best_t = None, None
        for e in per_eng:
            if idx[e] < len(per_eng[e]):
                l, v = per_eng[e][idx[e]]
                t_done = max(t_eng[e], lvl_ready[l]) + cost[e]
                if best_t is None or t_done < best_t:
                    best, best_t = e, t_done
        l, v = per_eng[best][idx[best]]
        idx[best] += 1
        t_eng[best] = best_t
        order.append((l, v, best))
    order.append((*SPLIT_PLANE, "S"))
    return order


_CACHED_NC = None


def _build_nc():
    """Build the per-core Bass program (identical on all 8 cores)."""
    nc = bacc.Bacc("TRN2", target_bir_lowering=False, debug=False,
                   num_devices=NCORES)

    act_thrs = sorted({v for (_, v) in ACT_PLANES} | {SPLIT_PLANE[1]})
    xt = nc.dram_tensor("xt", [IN, B], F32, kind="ExternalInput")
    consts = nc.dram_tensor("consts", [128, 2], F32, kind="ExternalInput")
    g = nc.dram_tensor("g", [IN, T_C], F16, kind="ExternalInput")
    qcols = nc.dram_tensor("qcols", [128, LEVELS * NTILE * NV], F16,
                           kind="ExternalInput")
    cvec = nc.dram_tensor("cvec", [1, OL], F32, kind="ExternalInput")
    y = nc.dram_tensor("y", [B, OL], F32, kind="ExternalOutput")

    NQ = 2  # psum chunks per level (8 t-tiles = 1024 cols each)

    with tile.TileContext(nc) as tc:
        with (
            tc.tile_pool(name="const", bufs=1) as cpool,
            tc.tile_pool(name="bits", bufs=1) as bpool,
            tc.tile_pool(name="codesb", bufs=1) as csbpool,
            tc.tile_pool(name="eq", bufs=14) as eqpool,
            tc.tile_pool(name="out", bufs=1) as opool,
            tc.tile_pool(name="psum_code", bufs=3,
                         space=bass.MemorySpace.PSUM) as pc,
            tc.tile_pool(name="psum_y", bufs=1,
                         space=bass.MemorySpace.PSUM) as py,
        ):
            xt_sb = cpool.tile([IN, B], F32, tag="xt")
            c_sb = cpool.tile([128, 2], F32, tag="consts")
            cv_sb = cpool.tile([1, OL], F32, tag="cvec")
            ones_sb = cpool.tile([1, B], F32, tag="ones")
            bias_sb = cpool.tile([128, max(1, len(act_thrs))], F32,
                                 tag="actbias")
            g_sb = [cpool.tile([IN, 512], F16, tag=f"g{q}", name=f"g_sb{q}")
                    for q in range(4)]
            # DMA issue order is SP-serialized (~500ns each): xt and g0
            # first (they gate the sign bits and the first code matmuls)
            nc.sync.dma_start(xt_sb[:], xt[:])
            nc.sync.dma_start(g_sb[0][:], g[:, 0:512])
            nc.sync.dma_start(c_sb[:], consts[:])
            for q in range(1, 4):
                nc.sync.dma_start(g_sb[q][:], g[:, q * 512:(q + 1) * 512])
            q_sb = cpool.tile([128, LEVELS * NTILE * NV], F16, tag="qcols")
            nc.sync.dma_start(q_sb[:], qcols[:])
            nc.sync.dma_start(cv_sb[:], cvec[:])
            nc.gpsimd.memset(ones_sb[:], 1.0)
            # ACT Sign biases are compile-time: memset, no DMA
            for i, v in enumerate(act_thrs):
                nc.gpsimd.memset(bias_sb[:, i:i + 1], -(float(v) - 0.5))

            # ---- sign bits (as fp16 0/1, j on partitions) ----
            bit1 = bpool.tile([IN, B], F16, tag="bit1")
            nc.vector.tensor_scalar(bit1[:], xt_sb[:], 0.0, None,
                                    mybir.AluOpType.is_ge)
            # rc = x - 2*m0*bit1   (== resid - m0)
            rc = bpool.tile([IN, B], F32, tag="rc")
            nc.vector.scalar_tensor_tensor(rc[:], bit1[:], c_sb[:, 0:1],
                                           xt_sb[:], mybir.AluOpType.mult,
                                           mybir.AluOpType.add)
            # bit2 = (rc >= -m0)
            bit2 = bpool.tile([IN, B], F16, tag="bit2")
            nc.vector.tensor_scalar(bit2[:], rc[:], c_sb[:, 1:2], None,
                                    mybir.AluOpType.is_ge)
            bits = [bit1, bit2]

            # ---- code matmuls + PSUM->SBUF fp16 copies (ACT) ----
            # codesb_l[t_p, (tile, b)], one [128, 2048] fp16 tensor per level
            codesb = [csbpool.tile([128, NTILE * B], F16, tag=f"code{l}",
                                   name=f"codesb{l}")
                      for l in range(LEVELS)]
            for l in range(LEVELS):
                for q in range(NQ):
                    cps = pc.tile([128, 8 * B], F32, tag="codepsum",
                                  name=f"cps{l}_{q}")
                    for k in range(8):
                        t_i = q * 8 + k
                        nc.tensor.matmul(
                            cps[:, k * B:(k + 1) * B],
                            g_sb[t_i // 4][:, (t_i % 4) * B:
                                           (t_i % 4 + 1) * B],
                            bits[l][:],
                            start=True, stop=True,
                        )
                    dst = codesb[l][:, q * 8 * B:(q + 1) * 8 * B]
                    nc.scalar.copy(dst, cps[:])

            # ---- step planes (DVE / ACT / POOL) + fused stage-B matmuls ----
            y_ps = py.tile([B, OL], F32, tag="ypsum")
            # rank-1 seed: y[b, o] = cvec[o]  (bias + const terms)
            nc.tensor.matmul(y_ps[:], ones_sb[:], cv_sb[:],
                             start=True, stop=False)
            order = _plane_emit_order()
            n_planes = len(order)
            n_dve_seen = 0
            n_pool_seen = 0
            H = NTILE * B // 2
            S = SPLIT_TILE * B
            for p_i, (l, v, eng) in enumerate(order):
                eq = eqpool.tile([128, NTILE * B], F16, tag="eq")
                thr = float(v) - 0.5
                if eng == "D":
                    n_dve_seen += 1
                    if l == 0 and n_dve_seen <= 3:
                        # first DVE planes: half-ops gated on individual
                        # code-copy chunks, so DVE starts a copy earlier
                        nc.vector.tensor_scalar(eq[:, 0:H], codesb[l][:, 0:H],
                                                thr, None,
                                                mybir.AluOpType.is_ge)
                        nc.vector.tensor_scalar(eq[:, H:], codesb[l][:, H:],
                                                thr, None,
                                                mybir.AluOpType.is_ge)
                    else:
                        nc.vector.tensor_scalar(eq[:], codesb[l][:], thr,
                                                None, mybir.AluOpType.is_ge)
                elif eng == "P":
                    n_pool_seen += 1
                    if n_pool_seen == 1:
                        # first Pool plane: half-ops for an earlier start
                        nc.gpsimd.tensor_scalar(eq[:, 0:H], codesb[l][:, 0:H],
                                                thr, None,
                                                mybir.AluOpType.is_ge)
                        nc.gpsimd.tensor_scalar(eq[:, H:], codesb[l][:, H:],
                                                thr, None,
                                                mybir.AluOpType.is_ge)
                    else:
                        nc.gpsimd.tensor_scalar(eq[:], codesb[l][:], thr,
                                                None, mybir.AluOpType.is_ge)
                elif eng == "A":  # ACT: sign(code - thr) in {-1, +1}
                    bcol = act_thrs.index(v)
                    nc.scalar.activation(eq[:], codesb[l][:],
                                         mybir.ActivationFunctionType.Sign,
                                         bias=bias_sb[:, bcol:bcol + 1])
                else:  # "S": final plane split ACT (sign) / POOL (is_ge)
                    bcol = act_thrs.index(v)
                    nc.scalar.activation(eq[:, 0:S], codesb[l][:, 0:S],
                                         mybir.ActivationFunctionType.Sign,
                                         bias=bias_sb[:, bcol:bcol + 1])
                    nc.gpsimd.tensor_scalar(eq[:, S:], codesb[l][:, S:],
                                            thr, None, mybir.AluOpType.is_ge)
                for t_i in range(NTILE):
                    col = (l * NTILE + t_i) * NV + (v - 1)
                    nc.tensor.matmul(
                        y_ps[:, t_i:t_i + 1],
                        eq[:, t_i * B:(t_i + 1) * B],
                        q_sb[:, col:col + 1],
                        start=False,
                        stop=(p_i == n_planes - 1),
                    )

            y_sb = opool.tile([B, OL], F32, tag="ysb")
            nc.vector.tensor_copy(y_sb[:], y_ps[:])
            nc.sync.dma_start(y[:], y_sb[:])

    nc.compile()
    return nc


def _host_prep(x, weight, bias, means):
    """Weight-static preprocessing: Q LUTs per level (fp64)."""
    w = weight.astype(np.float64)
    m = np.abs(means.astype(np.float64))
    cc = np.arange(KK)
    tt = (2 * ((cc[:, None] >> np.arange(K)[None, :]) & 1) - 1).astype(
        np.float64)          # [c, i]
    sig = tt                  # same construction for sign patterns [v, i]

    qs = []
    for l in range(LEVELS):
        # M[v, c] = prod_i (1 + m_l * sig[v,i] * tt[c,i]) / 2
        M = np.prod((1.0 + m[l] * sig[:, None, :] * tt[None, :, :]) * 0.5,
                    axis=-1)  # [v, c]
        q = w @ M.T           # [T, KK]
        qs.append(q)
    return qs


def _build_g(input_mask):
    G = np.zeros((IN, T), np.float64)
    cols = np.repeat(np.arange(T), K)
    vals = np.tile(2.0 ** np.arange(K), T)
    np.add.at(G, (input_mask.astype(np.int64), cols), vals)
    return G


def _make_in_maps(x, weight, bias, means, input_mask):
    qs = _host_prep(x, weight, bias, means)
    G = _build_g(input_mask)

    m0 = float(np.abs(means.astype(np.float64))[0])
    consts = np.zeros((128, 2), np.float32)
    consts[:, 0] = -2.0 * m0
    consts[:, 1] = -m0
    xt = np.ascontiguousarray(x.astype(np.float32).T)

    # step-basis coefficients: dq[t, v] = Q[t, v] - Q[t, v-1], v=1..15.
    # DVE/POOL planes are 0/1 steps (coeff dq); ACT planes are +-1 signs
    # (coeff dq/2, plus dq/2 folded into the constant).  The convention is
    # per (l, v, tile) since the split plane mixes engines across tiles.
    tile_of = (np.arange(T) % T_C) // 128    # core-local tile index [T]
    dqs, c0s = [], []
    for l in range(LEVELS):
        dq = np.diff(qs[l], axis=1)          # [T, 15]
        c0 = qs[l][:, 0].copy()              # [T]
        coeff = dq.copy()
        for v in range(1, KK):
            is_a = np.array([_plane_tile_engine(l, v, ti) == "A"
                             for ti in range(NTILE)])[tile_of]
            coeff[:, v - 1] = np.where(is_a, dq[:, v - 1] * 0.5,
                                       dq[:, v - 1])
            c0 += np.where(is_a, dq[:, v - 1] * 0.5, 0.0)
        dqs.append(coeff)
        c0s.append(c0)

    # const[o] = bias[o] + sum_l sum_j c0_l[o*IN+j]
    cvec_full = bias.astype(np.float64).copy()
    for l in range(LEVELS):
        cvec_full += c0s[l].reshape(OUT, IN).sum(-1)

    in_maps = []
    for c in range(NCORES):
        t0 = c * T_C
        gc = G[:, t0:t0 + T_C].astype(np.float16)
        # qcols[j, (l, tile, v-1)] = coeff_l[t0 + tile*128 + j, v]
        qc = np.empty((128, LEVELS, NTILE, NV), np.float16)
        for l in range(LEVELS):
            qc[:, l] = dqs[l][t0:t0 + T_C].reshape(
                NTILE, 128, NV).transpose(1, 0, 2)
        in_maps.append({
            "xt": xt,
            "consts": consts,
            "g": np.ascontiguousarray(gc),
            "qcols": np.ascontiguousarray(qc.reshape(128, -1)),
            "cvec": np.ascontiguousarray(
                cvec_full[c * OL:(c + 1) * OL].astype(np.float32)[None, :]),
        })
    return in_maps


_LAST_RESULTS = None


def kernel(x, weight, bias, means, input_mask):
    global _CACHED_NC, _LAST_RESULTS
    if _CACHED_NC is None:
        _CACHED_NC = _build_nc()
    nc = _CACHED_NC

    in_maps = _make_in_maps(x, weight, bias, means, input_mask)
    res = run_bass_kernel_spmd(nc, in_maps, list(range(NCORES)))
    _LAST_RESULTS = res
    out = np.concatenate([res.results[c]["y"] for c in range(NCORES)], axis=1)
    return out.astype(np.float32)


# revision 46
# speedup vs baseline: 1.0023x; 1.0023x over previous
"""
Trainium2 Bass kernel for nn_LinearLUT (residual-binarized LUT linear layer).

Math restructure
----------------
reference(x) computes, per sample b and per table t (t = o*128 + j, one table
per (out_feature o, in_feature j)):

  table_out[b,t] = sum_l f_t(m_l * s_l[b, idx_1(t)], ..., m_l * s_l[b, idx_4(t)])

where f_t is the multilinear (Lagrange) interpolation of the 16-entry LUT
weight[t, :] on {-1,+1}^4, s_l are the level-l sign bits of x, and
idx_i(t) = input_mask[t*4+i].  Since every argument is +-m_l, f_t only
depends on the 4 sign bits => precompute (host, weight-static):

  Q_l[t, v] = sum_c weight[t,c] * prod_i (1 + m_l*sig(v,i)*tt(c,i))/2

a 16-entry lookup per (t, level), indexed by the 4-bit sign code
  code_l[b,t] = sum_i 2^i * bit_l[b, idx_i(t)]  =  (bit_l @ G)[b,t]
with G[j,t] = sum_i 2^i [idx_i(t)==j]  -- ONE matmul per level.

Step-basis LUT evaluation (Abel summation):
  Q_l[t, c] = Q_l[t, 0] + sum_{v=1..15} dQ_l[t,v] * [c >= v]
so each basis plane [c >= v] is ONE instruction on any of three engines:
  DVE:  tensor_scalar is_ge               (0/1 plane, coeff dQ)
  ACT:  activation Sign, bias 0.5-v       (+-1 plane, coeff dQ/2, consts
                                           folded into cvec)
  POOL: tensor_scalar is_ge               (0/1 plane, coeff dQ)
The 30 planes (2 levels x 15 thresholds) are split across DVE/ACT/POOL to
balance engine busy time; the per-output-feature segment sum is fused into
PSUM-accumulated N=1 matmuls (j-contraction) as before.

On device (per core; tables sharded 8 ways, T_C=2048 tables = 16 out
features per core):
  1. sign bits from xT (DVE, 3 small ops)
  2. codeT[t_p,(tile,b)] = G_chunk^T @ bitT   (PE, 32 matmuls)
  3. PSUM->SBUF fp16 copies, [128,1024] x2 per level (ACT)
  4. step planes on DVE/ACT/POOL per the assignment
  5. LUT-eval + segment-sum fused into PE: y[b,o] += plane^T @ dq_col
     (N=1 matmuls, PSUM-accumulated; seeded by rank-1 cvec matmul)
  6. y PSUM->SBUF on DVE, DMA out [128, 16] f32; host concatenates cores.
"""

import numpy as np

import concourse.bass as bass
import concourse.bacc as bacc
import concourse.mybir as mybir
import concourse.tile as tile
from concourse.bass_utils import run_bass_kernel_spmd

# Problem dims (hardcoded per contract)
LEVELS = 2
K = 4
KK = 16
IN = 128
OUT = 128
B = 128
T = IN * OUT  # 16384
NCORES = 8
T_C = T // NCORES     # 2048 tables per core
OL = OUT // NCORES    # 16 out features per core
NTILE = T_C // 128    # 16 t-tiles per core
NV = KK - 1           # 15 step thresholds v=1..15 (v=0 folded into cvec)

F16 = mybir.dt.float16
F32 = mybir.dt.float32

# ---- plane -> engine assignment (tuned against the CoreSim cost model) ----
# 'D' = DVE is_ge (0/1), 'A' = ACT Sign (+-1, coeff/2), 'P' = POOL is_ge.
ACT_PLANES = {(0, 10), (0, 11), (1, 12), (1, 13)}
POOL_PLANES = {(0, 12), (0, 13), (0, 14), (0, 15), (1, 14), (1, 15)}
# final plane: split three ways (DVE / ACT / POOL by tile range) so all
# three engines finish together
SPLIT_PLANE = (1, 11)
SPLIT_D = 5   # tiles [0, 5) on DVE
SPLIT_A = 9   # tiles [5, 9) on ACT; rest on POOL


def _plane_tile_engine(l, v, tile):
    """Engine that computes basis plane (l, v) for tile `tile`."""
    if (l, v) == SPLIT_PLANE:
        if tile < SPLIT_D:
            return "D"
        return "A" if tile < SPLIT_A else "P"
    if (l, v) in ACT_PLANES:
        return "A"
    if (l, v) in POOL_PLANES:
        return "P"
    return "D"


def _plane_emit_order():
    """Interleave (l, v, engine) so each engine's queue is busy end-to-end
    and PE consumes planes roughly in completion order.  Each engine's own
    planes are emitted level-0 first (level-1 code is ready later)."""
    per_eng = {"D": [], "A": [], "P": []}
    for l in range(LEVELS):
        for v in range(1, KK):
            if (l, v) == SPLIT_PLANE:
                continue  # emitted last, split across ACT and POOL
            per_eng[_plane_tile_engine(l, v, 0)].append((l, v))
    # estimated per-plane engine cost (ns) and start offsets
    cost = {"D": 594.0, "A": 1892.0, "P": 1707.0}
    start = {"D": 4580.0, "A": 7700.0, "P": 4680.0}
    lvl_ready = {0: 4580.0, 1: 7700.0}
    order = []
    t_eng = dict(start)
    idx = {e: 0 for e in per_eng}
    while any(idx[e] < len(per_eng[e]) for e in per_eng):
        # pick engine whose next plane completes earliest
        best, best_t = None, None
        for e in per_eng:
            if idx[e] < len(per_eng[e]):
                l, v = per_eng[e][idx[e]]
                t_done = max(t_eng[e], lvl_ready[l]) + cost[e]
                if best_t is None or t_done < best_t:
                    best, best_t = e, t_done
        l, v = per_eng[best][idx[best]]
        idx[best] += 1
        t_eng[best] = best_t
        order.append((l, v, best))
    order.append((*SPLIT_PLANE, "S"))
    return order


_CACHED_NC = None


def _build_nc():
    """Build the per-core Bass program (identical on all 8 cores)."""
    nc = bacc.Bacc("TRN2", target_bir_lowering=False, debug=False,
                   num_devices=NCORES)

    act_thrs = sorted({v for (_, v) in ACT_PLANES} | {SPLIT_PLANE[1]})
    xt = nc.dram_tensor("xt", [IN, B], F32, kind="ExternalInput")
    consts = nc.dram_tensor("consts", [128, 2], F32, kind="ExternalInput")
    g = nc.dram_tensor("g", [IN, T_C], F16, kind="ExternalInput")
    qcols = nc.dram_tensor("qcols", [128, LEVELS * NTILE * NV], F16,
                           kind="ExternalInput")
    cvec = nc.dram_tensor("cvec", [1, OL], F32, kind="ExternalInput")
    y = nc.dram_tensor("y", [B, OL], F32, kind="ExternalOutput")

    NQ = 2  # psum chunks per level (8 t-tiles = 1024 cols each)

    with tile.TileContext(nc) as tc:
        with (
            tc.tile_pool(name="const", bufs=1) as cpool,
            tc.tile_pool(name="bits", bufs=1) as bpool,
            tc.tile_pool(name="codesb", bufs=1) as csbpool,
            tc.tile_pool(name="eq", bufs=14) as eqpool,
            tc.tile_pool(name="out", bufs=1) as opool,
            tc.tile_pool(name="psum_code", bufs=3,
                         space=bass.MemorySpace.PSUM) as pc,
            tc.tile_pool(name="psum_code1", bufs=2,
                         space=bass.MemorySpace.PSUM) as pc1,
            tc.tile_pool(name="psum_y", bufs=1,
                         space=bass.MemorySpace.PSUM) as py,
        ):
            xt_sb = cpool.tile([IN, B], F32, tag="xt")
            c_sb = cpool.tile([128, 2], F32, tag="consts")
            cv_sb = cpool.tile([1, OL], F32, tag="cvec")
            ones_sb = cpool.tile([1, B], F32, tag="ones")
            bias_sb = cpool.tile([128, max(1, len(act_thrs))], F32,
                                 tag="actbias")
            g_sb = [cpool.tile([IN, 512], F16, tag=f"g{q}", name=f"g_sb{q}")
                    for q in range(4)]
            # DMA issue order is SP-serialized (~500ns each): xt and g0
            # first (they gate the sign bits and the first code matmuls)
            nc.sync.dma_start(xt_sb[:], xt[:])
            nc.sync.dma_start(g_sb[0][:], g[:, 0:512])
            nc.sync.dma_start(c_sb[:], consts[:])
            for q in range(1, 4):
                nc.sync.dma_start(g_sb[q][:], g[:, q * 512:(q + 1) * 512])
            q_sb = cpool.tile([128, LEVELS * NTILE * NV], F16, tag="qcols")
            nc.sync.dma_start(q_sb[:], qcols[:])
            nc.sync.dma_start(cv_sb[:], cvec[:])
            nc.gpsimd.memset(ones_sb[:], 1.0)
            # ACT Sign biases are compile-time: memset, no DMA
            for i, v in enumerate(act_thrs):
                nc.gpsimd.memset(bias_sb[:, i:i + 1], -(float(v) - 0.5))

            # ---- sign bits (as fp16 0/1, j on partitions) ----
            bit1 = bpool.tile([IN, B], F16, tag="bit1")
            nc.vector.tensor_scalar(bit1[:], xt_sb[:], 0.0, None,
                                    mybir.AluOpType.is_ge)
            # rc = x - 2*m0*bit1   (== resid - m0)
            rc = bpool.tile([IN, B], F32, tag="rc")
            nc.vector.scalar_tensor_tensor(rc[:], bit1[:], c_sb[:, 0:1],
                                           xt_sb[:], mybir.AluOpType.mult,
                                           mybir.AluOpType.add)
            # bit2 = (rc >= -m0)
            bit2 = bpool.tile([IN, B], F16, tag="bit2")
            nc.vector.tensor_scalar(bit2[:], rc[:], c_sb[:, 1:2], None,
                                    mybir.AluOpType.is_ge)
            bits = [bit1, bit2]

            # ---- code matmuls + PSUM->SBUF fp16 copies (ACT) ----
            # codesb_l[t_p, (tile, b)], one [128, 2048] fp16 tensor per level.
            # Level 0 is chunked 4x[128,512] (each gated on a single g DMA)
            # so the first copy starts right after the first 4 matmuls;
            # level 1 is chunked 2x[128,1024].
            codesb = [csbpool.tile([128, NTILE * B], F16, tag=f"code{l}",
                                   name=f"codesb{l}")
                      for l in range(LEVELS)]
            chunks = [(0, q, 4) for q in range(4)] + [(1, q, 8) for q in
                                                      range(2)]
            for l, q, w in chunks:
                cps = (pc if w == 4 else pc1).tile(
                    [128, w * B], F32, tag=f"codepsum{w}",
                    name=f"cps{l}_{q}")
                for k in range(w):
                    t_i = q * w + k
                    nc.tensor.matmul(
                        cps[:, k * B:(k + 1) * B],
                        g_sb[t_i // 4][:, (t_i % 4) * B:(t_i % 4 + 1) * B],
                        bits[l][:],
                        start=True, stop=True,
                    )
                dst = codesb[l][:, q * w * B:(q + 1) * w * B]
                nc.scalar.copy(dst, cps[:])

            # ---- step planes (DVE / ACT / POOL) + fused stage-B matmuls ----
            y_ps = py.tile([B, OL], F32, tag="ypsum")
            # rank-1 seed: y[b, o] = cvec[o]  (bias + const terms)
            nc.tensor.matmul(y_ps[:], ones_sb[:], cv_sb[:],
                             start=True, stop=False)
            order = _plane_emit_order()
            n_planes = len(order)
            H = NTILE * B // 2


            # --- head: first 3 DVE level-0 planes as an interleaved
            # quarter-ladder, each quarter gated on one [128,512] copy chunk,
            # with the matmuls for that quarter's tiles right behind ---
            head = [(p_i, l, v) for p_i, (l, v, e) in enumerate(order)
                    if e == "D" and l == 0][:3]
            head_ids = {p_i for p_i, _, _ in head}
            head_eq = {}
            for p_i, l, v in head:
                head_eq[p_i] = eqpool.tile([128, NTILE * B], F16, tag="eq",
                                           name=f"eq_head{p_i}")
            QW = 4 * B
            for quarter in range(4):
                for p_i, l, v in head:
                    eq = head_eq[p_i]
                    nc.vector.tensor_scalar(
                        eq[:, quarter * QW:(quarter + 1) * QW],
                        codesb[l][:, quarter * QW:(quarter + 1) * QW],
                        float(v) - 0.5, None, mybir.AluOpType.is_ge)
                for p_i, l, v in head:
                    eq = head_eq[p_i]
                    for t_i in range(quarter * 4, (quarter + 1) * 4):
                        col = (l * NTILE + t_i) * NV + (v - 1)
                        nc.tensor.matmul(
                            y_ps[:, t_i:t_i + 1],
                            eq[:, t_i * B:(t_i + 1) * B],
                            q_sb[:, col:col + 1],
                            start=False, stop=False,
                        )

            n_pool_seen = 0
            for p_i, (l, v, eng) in enumerate(order):
                if p_i in head_ids:
                    continue
                eq = eqpool.tile([128, NTILE * B], F16, tag="eq")
                thr = float(v) - 0.5
                if eng == "D":
                    nc.vector.tensor_scalar(eq[:], codesb[l][:], thr,
                                            None, mybir.AluOpType.is_ge)
                elif eng == "P":
                    n_pool_seen += 1
                    if n_pool_seen == 1:
                        # first Pool plane: half-ops for an earlier start
                        nc.gpsimd.tensor_scalar(eq[:, 0:H], codesb[l][:, 0:H],
                                                thr, None,
                                                mybir.AluOpType.is_ge)
                        nc.gpsimd.tensor_scalar(eq[:, H:], codesb[l][:, H:],
                                                thr, None,
                                                mybir.AluOpType.is_ge)
                    else:
                        nc.gpsimd.tensor_scalar(eq[:], codesb[l][:], thr,
                                                None, mybir.AluOpType.is_ge)
                elif eng == "A":  # ACT: sign(code - thr) in {-1, +1}
                    bcol = act_thrs.index(v)
                    nc.scalar.activation(eq[:], codesb[l][:],
                                         mybir.ActivationFunctionType.Sign,
                                         bias=bias_sb[:, bcol:bcol + 1])
                else:  # "S": final plane split DVE / ACT (sign) / POOL
                    bcol = act_thrs.index(v)
                    SD, SA = SPLIT_D * B, SPLIT_A * B
                    nc.vector.tensor_scalar(eq[:, 0:SD], codesb[l][:, 0:SD],
                                            thr, None, mybir.AluOpType.is_ge)
                    nc.scalar.activation(eq[:, SD:SA], codesb[l][:, SD:SA],
                                         mybir.ActivationFunctionType.Sign,
                                         bias=bias_sb[:, bcol:bcol + 1])
                    nc.gpsimd.tensor_scalar(eq[:, SA:], codesb[l][:, SA:],
                                            thr, None, mybir.AluOpType.is_ge)
                for t_i in range(NTILE):
                    col = (l * NTILE + t_i) * NV + (v - 1)
                    nc.tensor.matmul(
                        y_ps[:, t_i:t_i + 1],
                        eq[:, t_i * B:(t_i + 1) * B],
                        q_sb[:, col:col + 1],
                        start=False,
                        stop=(p_i == n_planes - 1),
                    )

            y_sb = opool.tile([B, OL], F32, tag="ysb")
            nc.vector.tensor_copy(y_sb[:], y_ps[:])
            nc.sync.dma_start(y[:], y_sb[:])

    nc.compile()
    return nc


def _host_prep(x, weight, bias, means):
    """Weight-static preprocessing: Q LUTs per level (fp64)."""
    w = weight.astype(np.float64)
    m = np.abs(means.astype(np.float64))
    cc = np.arange(KK)
    tt = (2 * ((cc[:, None] >> np.arange(K)[None, :]) & 1) - 1).astype(
        np.float64)          # [c, i]
    sig = tt                  # same construction for sign patterns [v, i]

    qs = []
    for l in range(LEVELS):
        # M[v, c] = prod_i (1 + m_l * sig[v,i] * tt[c,i]) / 2
        M = np.prod((1.0 + m[l] * sig[:, None, :] * tt[None, :, :]) * 0.5,
                    axis=-1)  # [v, c]
        q = w @ M.T           # [T, KK]
        qs.append(q)
    return qs


def _build_g(input_mask):
    G = np.zeros((IN, T), np.float64)
    cols = np.repeat(np.arange(T), K)
    vals = np.tile(2.0 ** np.arange(K), T)
    np.add.at(G, (input_mask.astype(np.int64), cols), vals)
    return G


def _make_in_maps(x, weight, bias, means, input_mask):
    qs = _host_prep(x, weight, bias, means)
    G = _build_g(input_mask)

    m0 = float(np.abs(means.astype(np.float64))[0])
    consts = np.zeros((128, 2), np.float32)
    consts[:, 0] = -2.0 * m0
    consts[:, 1] = -m0
    xt = np.ascontiguousarray(x.astype(np.float32).T)

    # step-basis coefficients: dq[t, v] = Q[t, v] - Q[t, v-1], v=1..15.
    # DVE/POOL planes are 0/1 steps (coeff dq); ACT planes are +-1 signs
    # (coeff dq/2, plus dq/2 folded into the constant).  The convention is
    # per (l, v, tile) since the split plane mixes engines across tiles.
    tile_of = (np.arange(T) % T_C) // 128    # core-local tile index [T]
    dqs, c0s = [], []
    for l in range(LEVELS):
        dq = np.diff(qs[l], axis=1)          # [T, 15]
        c0 = qs[l][:, 0].copy()              # [T]
        coeff = dq.copy()
        for v in range(1, KK):
            is_a = np.array([_plane_tile_engine(l, v, ti) == "A"
                             for ti in range(NTILE)])[tile_of]
            coeff[:, v - 1] = np.where(is_a, dq[:, v - 1] * 0.5,
                                       dq[:, v - 1])
            c0 += np.where(is_a, dq[:, v - 1] * 0.5, 0.0)
        dqs.append(coeff)
        c0s.append(c0)

    # const[o] = bias[o] + sum_l sum_j c0_l[o*IN+j]
    cvec_full = bias.astype(np.float64).copy()
    for l in range(LEVELS):
        cvec_full += c0s[l].reshape(OUT, IN).sum(-1)

    in_maps = []
    for c in range(NCORES):
        t0 = c * T_C
        gc = G[:, t0:t0 + T_C].astype(np.float16)
        # qcols[j, (l, tile, v-1)] = coeff_l[t0 + tile*128 + j, v]
        qc = np.empty((128, LEVELS, NTILE, NV), np.float16)
        for l in range(LEVELS):
            qc[:, l] = dqs[l][t0:t0 + T_C].reshape(
                NTILE, 128, NV).transpose(1, 0, 2)
        in_maps.append({
            "xt": xt,
            "consts": consts,
            "g": np.ascontiguousarray(gc),
            "qcols": np.ascontiguousarray(qc.reshape(128, -1)),
            "cvec": np.ascontiguousarray(
                cvec_full[c * OL:(c + 1) * OL].astype(np.float32)[None, :]),
        })
    return in_maps


_LAST_RESULTS = None


def kernel(x, weight, bias, means, input_mask):
    global _CACHED_NC, _LAST_RESULTS
    if _CACHED_NC is None:
        _CACHED_NC = _build_nc()
    nc = _CACHED_NC

    in_maps = _make_in_maps(x, weight, bias, means, input_mask)
    res = run_bass_kernel_spmd(nc, in_maps, list(range(NCORES)))
    _LAST_RESULTS = res
    out = np.concatenate([res.results[c]["y"] for c in range(NCORES)], axis=1)
    return out.astype(np.float32)
